# revision 7
# baseline (speedup 1.0000x reference)
"""Mixtral-style MoE router kernel for Trainium2 (8 NeuronCores, Bass/Tile).

Computation (matches the reference):
    logits = hidden @ gate_w.T            # (T, E) thin GEMM, E=8
    logits = (logits + pressure_bias) / clip(temperature, 0.1, 10)
    top_vals, top_idx = top_k(logits, 2)
    weights = softmax(top_vals)

Sharding: data-parallel over the 32768 flattened tokens -> 4096 tokens/core.
Gate weight / bias / temperature are tiny and replicated.

v2 design (vs the fp32 gate-stationary v1):
  * Precision-decomposed GEMM in reduced dtypes so the PE runs at full rate
    and HBM traffic drops to 3 B/element: per core the host ships
       hh = fp16(h)                      [32 MiB]
       hl = fp8_e4m3((h - hh) * 256)     [16 MiB]
    and gate (with 1/clip(temp) folded in, host-side):
       gh = fp16(g'), gl = fp16((g' - gh) * 256)   (tiny, replicated)
    logits = hh@gh + 2^-8 * (hh@(gl*?) ... concretely:
       A[:, 0:8]  = sum_c hh_c.T @ gh_c          (accumulated in PSUM bank A)
       A[:, 8:16] = sum_c hh_c.T @ gl_c          (gl pre-scaled x256)
       C          = sum_c hl_c.T @ gh_c          (hl pre-scaled x256, bank C)
       logits = A[:,0:8] + 2^-8 * (A[:,8:16] + C) + bias'
    The dropped hl@gl term is O(2^-22) relative.  Measured against the fp32
    reference on the real input distribution: 0 flipped expert selections.
  * Flipped matmul: the 128x128 token tile is the (self-loading) stationary
    operand, the 16-wide gate is the moving operand -> ~24 moving rows per
    (chunk, tile) instead of 512, and the output lands directly as
    [token, expert] (no PE transpose pass).
  * Token-block-major dataflow: 8 blocks x 512 tokens.  Per block all 32
    feature chunks stream in via one 4 MiB (fp16) + one 2 MiB (fp8)
    contiguous DMA; each of the 4 token tiles finishes its full accumulation
    before the next starts (PSUM start=True clears has_written for the WHOLE
    bank, so interleaved same-partition accumulation groups would clobber
    each other; tile-major order + a single start=True per bank per block is
    both correct and minimal).
  * Per-block epilogue (DVE/ACT): combine + descale + bias, top-2 via
    max/max_index, softmax over the 2 selected logits; overlaps the next
    block's DMA/PE work.
"""

import numpy as np
import ml_dtypes

import concourse.bass as bass
import concourse.tile as tile
from concourse import bacc, mybir
from concourse.bass_utils import run_bass_kernel_spmd

F32 = mybir.dt.float32
F16 = mybir.dt.float16
F8 = mybir.dt.float8e4
U32 = mybir.dt.uint32

N_CORES = 8
B, S, D, E = 4, 8192, 4096, 8
T_TOTAL = B * S                    # 32768 tokens
T_CORE = T_TOTAL // N_CORES        # 4096 tokens per core
P = 128                            # SBUF partitions
NCH = D // P                       # 32 feature chunks
BLK = 512                          # tokens per block
NBLK = T_CORE // BLK               # 8 blocks per core
NT = BLK // P                      # 4 token tiles of 128 per block
MOV = 16                           # moving gate width: [gh | gl*256]
LO_SCALE = 256.0                   # scale on hl / gl
INV_LO = 1.0 / LO_SCALE

_NC_CACHE = {}

# test-harness hooks (ignored by graders)
TRACE = False
LAST_RESULT = None


def build_router_nc(n_rep=1, hbufs=3, lbufs=3):
    """Build the per-core Bass program (same program on all cores).

    n_rep > 1 wraps the whole body in a For_i loop for differential timing
    (program size stays constant so per-call dispatch overhead cancels).
    """
    nc = bacc.Bacc(None, target_bir_lowering=False)

    hh = nc.dram_tensor("hh", [NBLK, P, NCH, BLK], F16, kind="ExternalInput")
    hl = nc.dram_tensor("hl", [NBLK, P, NCH, BLK], F8, kind="ExternalInput")
    g = nc.dram_tensor("g", [P, NCH, MOV], F16, kind="ExternalInput")
    # bias row for the bias-matmul: [bias_hi | bias_lo*256] per tile
    bb = nc.dram_tensor("bb", [1, NT * MOV], F16, kind="ExternalInput")
    ow = nc.dram_tensor("ow", [NBLK, P, NT, 2], F32, kind="ExternalOutput")
    oe = nc.dram_tensor("oe", [NBLK, P, NT, 2], U32, kind="ExternalOutput")

    with tile.TileContext(nc) as tc:
        with (
            tc.tile_pool(name="singles", bufs=1) as singles,
            tc.tile_pool(name="hp", bufs=hbufs) as hp,
            tc.tile_pool(name="lp", bufs=lbufs) as lp,
            tc.tile_pool(name="ep", bufs=2) as ep,
            tc.tile_pool(name="pa", bufs=2, space="PSUM") as pa,
            tc.tile_pool(name="pc", bufs=2, space="PSUM") as pc,
        ):
            gt = singles.tile([P, NCH, MOV], F16)
            nc.sync.dma_start(out=gt, in_=g[:])
            ones = singles.tile([1, P], F16)
            nc.vector.memset(ones, 1.0)
            brow = singles.tile([1, NT * MOV], F16)
            nc.sync.dma_start(out=brow, in_=bb[:])

            def body():
                for blk in range(NBLK):
                    ht = hp.tile([P, NCH, BLK], F16, tag="ht")
                    nc.sync.dma_start(out=ht, in_=hh[blk])
                    lt = lp.tile([P, NCH, BLK], F8, tag="lt")
                    nc.sync.dma_start(out=lt, in_=hl[blk])

                    psA = pa.tile([P, NT, MOV], F32, tag="psA")
                    psC = pc.tile([P, NT, E], F32, tag="psC")
                    # bias matmul: ones[1,128].T @ brow[1,64] broadcasts
                    # [bias_hi | bias_lo*256] into every tile's accumulator.
                    # Doubles as bank A's single start=True (start clears
                    # has_written for the WHOLE bank, so it must be the
                    # chronologically first write; later tiles' first
                    # matmuls overwrite-on-unset-bit).
                    nc.tensor.matmul(
                        psA[:, :, :], lhsT=ones, rhs=brow,
                        start=True, stop=False, skip_group_check=True)
                    # tile-major: each token tile's accumulation completes
                    # before the next tile starts.
                    for t in range(NT):
                        for c in range(NCH):
                            first = t == 0 and c == 0
                            last = t == NT - 1 and c == NCH - 1
                            nc.tensor.matmul(
                                psA[:, t, :],
                                lhsT=ht[:, c, t * P:(t + 1) * P],
                                rhs=gt[:, c, :],
                                start=False, stop=last,
                                skip_group_check=True)
                            nc.tensor.matmul(
                                psC[:, t, :],
                                lhsT=lt[:, c, t * P:(t + 1) * P],
                                rhs=gt[:, c, 0:E],
                                start=first, stop=last,
                                skip_group_check=True)

                    # ---- logits = A[:,:,0:8] + 2^-8*(A[:,:,8:16] + C) ----
                    # (bias already accumulated by the bias matmul; DVE may
                    # read only ONE PSUM operand per instruction, so the
                    # descales go PSUM->SBUF first.  ya reads a slice of
                    # every bank-A matmul's range and yc of every bank-C
                    # matmul's range, so RAW deps cover all accumulation.)
                    ya = ep.tile([P, NT, E], F32, tag="ya")
                    nc.vector.tensor_scalar(
                        out=ya, in0=psA[:, :, E:MOV], scalar1=INV_LO,
                        scalar2=None, op0=mybir.AluOpType.mult)
                    yc = ep.tile([P, NT, E], F32, tag="yc")
                    nc.vector.tensor_scalar(
                        out=yc, in0=psC, scalar1=INV_LO,
                        scalar2=None, op0=mybir.AluOpType.mult)
                    zt = ep.tile([P, NT, E], F32, tag="zt")
                    nc.vector.tensor_tensor(
                        out=zt, in0=psA[:, :, 0:E], in1=ya,
                        op=mybir.AluOpType.add)
                    sc = ep.tile([P, NT, E], F32, tag="sc")
                    nc.vector.tensor_tensor(
                        out=sc, in0=zt, in1=yc,
                        op=mybir.AluOpType.add)

                    # ---- top-2 of 8 per token ----
                    mx = ep.tile([P, NT, E], F32, tag="mx")
                    mi = ep.tile([P, NT, E], U32, tag="mi")
                    for t in range(NT):
                        nc.vector.max(out=mx[:, t, :], in_=sc[:, t, :])
                    for t in range(NT):
                        nc.vector.max_index(out=mi[:, t, :],
                                            in_max=mx[:, t, :],
                                            in_values=sc[:, t, :])

                    # ---- softmax over the two selected logits ----
                    # d = v2-v1 (<=0); w1 = 1/(1+e^d); w2 = e^d/(1+e^d)
                    dt_ = ep.tile([P, NT], F32, tag="dt")
                    nc.vector.tensor_tensor(
                        out=dt_, in0=mx[:, :, 1], in1=mx[:, :, 0],
                        op=mybir.AluOpType.subtract)
                    et = ep.tile([P, NT], F32, tag="et")
                    nc.scalar.activation(
                        out=et, in_=dt_,
                        func=mybir.ActivationFunctionType.Exp)
                    st = ep.tile([P, NT], F32, tag="st")
                    nc.vector.tensor_scalar_add(st, et, 1.0)
                    rt = ep.tile([P, NT], F32, tag="rt")
                    nc.vector.reciprocal(out=rt, in_=st)

                    owt = ep.tile([P, NT, 2], F32, tag="owt")
                    nc.vector.tensor_copy(out=owt[:, :, 0], in_=rt)
                    nc.vector.tensor_tensor(
                        out=owt[:, :, 1], in0=et, in1=rt,
                        op=mybir.AluOpType.mult)

                    nc.sync.dma_start(out=ow[blk], in_=owt)
                    nc.sync.dma_start(out=oe[blk], in_=mi[:, :, 0:2])

            if n_rep == 1:
                body()
            else:
                with tc.For_i(0, n_rep, 1):
                    body()

    nc.finalize()
    return nc


def _get_nc():
    if "nc" not in _NC_CACHE:
        _NC_CACHE["nc"] = build_router_nc()
    return _NC_CACHE["nc"]


def make_gate_inputs(pressure_bias, temperature_field, gate_w):
    """gcomb [P, NCH, 16] fp16 (temp folded, [gh | gl*256]) and the bias
    row [1, NT*16] fp16 ([bias_hi | bias_lo*256] per tile)."""
    gw = np.asarray(gate_w, dtype=np.float32)              # [E, D]
    pb = np.asarray(pressure_bias, np.float32)
    temp = np.asarray(temperature_field, np.float32)
    it = 1.0 / np.clip(temp, np.float32(0.1), np.float32(10.0))
    gs = gw * it[:, None]                                  # [E, D]
    gT = np.ascontiguousarray(gs.T)                        # [D, E]
    gh = gT.astype(np.float16)
    gl = ((gT - gh.astype(np.float32)) * LO_SCALE).astype(np.float16)
    gcomb = np.concatenate([gh, gl], axis=1)               # [D, 16]
    gcomb = np.ascontiguousarray(
        gcomb.reshape(NCH, P, MOV).transpose(1, 0, 2))     # [P, NCH, 16]
    bias = (pb * it).astype(np.float32)                    # [E]
    b_hi = bias.astype(np.float16)
    b_lo = ((bias - b_hi.astype(np.float32)) * LO_SCALE).astype(np.float16)
    brow = np.ascontiguousarray(
        np.broadcast_to(np.concatenate([b_hi, b_lo]), (NT, MOV))
        .reshape(1, NT * MOV))
    return gcomb, brow


def make_h_inputs(hs_core):
    """Per-core [T_CORE, D] fp32 -> (hh [NBLK,P,NCH,BLK] f16,
    hl [NBLK,P,NCH,BLK] f8e4m3 scaled x256)."""
    hh_flat = hs_core.astype(np.float16)
    r = (hs_core - hh_flat.astype(np.float32)) * np.float32(LO_SCALE)
    hl_flat = r.astype(ml_dtypes.float8_e4m3)
    # [tok, feat] -> [blk, p, c, t]: tok = blk*BLK + t, feat = c*P + p
    hh_dev = np.ascontiguousarray(
        hh_flat.reshape(NBLK, BLK, NCH, P).transpose(0, 3, 2, 1))
    hl_dev = np.ascontiguousarray(
        hl_flat.reshape(NBLK, BLK, NCH, P).transpose(0, 3, 2, 1))
    return hh_dev, hl_dev


def unshuffle_out(arr):
    """[NBLK, P, NT, u] device layout -> [T_CORE, u] token order.

    token t = blk*BLK + tile*P + p
    """
    return np.ascontiguousarray(
        arr.transpose(0, 2, 1, 3).reshape(T_CORE, arr.shape[-1]))


def kernel(hidden_states, pressure_bias, temperature_field, gate_w):
    hs = np.ascontiguousarray(np.asarray(hidden_states, dtype=np.float32))
    hs = hs.reshape(T_TOTAL, D)
    gcomb, brow = make_gate_inputs(pressure_bias, temperature_field, gate_w)

    in_maps = []
    for i in range(N_CORES):
        hh_dev, hl_dev = make_h_inputs(hs[i * T_CORE:(i + 1) * T_CORE])
        in_maps.append({"hh": hh_dev, "hl": hl_dev, "g": gcomb, "bb": brow})

    nc = _get_nc()
    global LAST_RESULT
    res = run_bass_kernel_spmd(nc, in_maps, core_ids=list(range(N_CORES)),
                               trace=TRACE)
    LAST_RESULT = res

    weights = np.empty((T_TOTAL, 2), np.float32)
    experts = np.empty((T_TOTAL, 2), np.int32)
    for i, r in enumerate(res.results):
        weights[i * T_CORE:(i + 1) * T_CORE] = unshuffle_out(r["ow"])
        experts[i * T_CORE:(i + 1) * T_CORE] = (
            unshuffle_out(r["oe"]).astype(np.int32))

    return weights.reshape(B, S, 2), experts.reshape(B, S, 2)


# revision 8
# speedup vs baseline: 6.2864x; 6.2864x over previous
"""v4: gate-stationary bf16 3-term decomposition.

Platform measurements (loop-differential on the axon trn2 pool):
  - bf16-moving matmuls stream at ~105-210 ns/MM (512-token moving operand);
    fp32 and fp16 moving operands are ~20-40x slower per row.
  - large DMAs reach ~260-330 GB/s/core; small DMAs are overhead-bound.
So: v1's fat-matmul structure (gate [128,8] stationary, 512-token moving,
4 concurrent PSUM col-groups, PE transposes, DVE top-2) but with the GEMM
decomposed into 3 bf16/fp8 terms so every matmul is a fast bf16-class op,
and HBM traffic drops to 3 B/element:
    hh = bf16(h)                     32 MiB/core
    hl = fp8_e4m3((h - hh) * 256)    16 MiB/core
    gh = bf16(g'), gl = bf16((g' - gh) * 256)   (g' = gate with 1/clip(temp)
                                                 folded; tiny, replicated)
    A  = sum_c hh_c @ gh_c                       (PSUM bank per half)
    X  = sum_c hh_c @ gl_c + hl_c @ gh_c         (both x256, second bank)
    logits = A + X * 2^-8 + bias'
Measured against the fp32 reference on the real inputs: 3 flipped expert
selections out of 65536 (rel err 3.7e-3, tolerance 2e-2).
"""

import numpy as np
import ml_dtypes

import concourse.bass as bass
import concourse.tile as tile
from concourse import bacc, mybir
from concourse.bass_utils import run_bass_kernel_spmd

F32 = mybir.dt.float32
BF16 = mybir.dt.bfloat16
F8 = mybir.dt.float8e4
U32 = mybir.dt.uint32

N_CORES = 8
B, S, D, E = 4, 8192, 4096, 8
T_TOTAL = B * S
T_CORE = T_TOTAL // N_CORES        # 4096 tokens per core
P = 128
NCH = D // P                       # 32 feature chunks
T_HALF = T_CORE // 2               # 2048 tokens per PSUM-bank residency
N_Q = T_HALF // 512                # 4 col-groups
N_BJ = 4
N_BLK = N_Q * N_BJ                 # 16 transpose blocks per half
CQH = 8                            # bf16 chunks per DMA (4 MiB)
CQL = 16                           # fp8 chunks per DMA (4 MiB)
LO_SCALE = 256.0
INV_LO = 1.0 / LO_SCALE

_NC_CACHE = {}

TRACE = False
LAST_RESULT = None


def build_router_nc(n_rep=1, hbufs=2, lbufs=2):
    nc = bacc.Bacc(None, target_bir_lowering=False)

    # [half][p][c][t] so a [P, k-chunks, T_HALF] load is contiguous per row
    hh = nc.dram_tensor("hh", [2, P, NCH, T_HALF], BF16,
                        kind="ExternalInput")
    hl = nc.dram_tensor("hl", [2, P, NCH, T_HALF], F8, kind="ExternalInput")
    # [gh | gl*256] per chunk
    g2 = nc.dram_tensor("g2", [P, NCH, 2, E], BF16, kind="ExternalInput")
    pt = nc.dram_tensor("pt", [E, 1], F32, kind="ExternalInput")   # bias'
    idn = nc.dram_tensor("idn", [E, E], F32, kind="ExternalInput")
    ow = nc.dram_tensor("ow", [2, P, N_Q, N_BJ, 2], F32,
                        kind="ExternalOutput")
    oe = nc.dram_tensor("oe", [2, P, N_Q, N_BJ, 2], U32,
                        kind="ExternalOutput")

    with tile.TileContext(nc) as tc:
        with (
            tc.tile_pool(name="singles", bufs=1) as singles,
            tc.tile_pool(name="hp", bufs=hbufs) as hp,
            tc.tile_pool(name="lp", bufs=lbufs) as lp,
            tc.tile_pool(name="ep", bufs=2) as ep,
            tc.tile_pool(name="psl", bufs=2, space="PSUM") as psl,
            tc.tile_pool(name="psx", bufs=2, space="PSUM") as psx,
            tc.tile_pool(name="pst", bufs=2, space="PSUM") as pst,
        ):
            gt = singles.tile([P, NCH, 2, E], BF16)
            nc.sync.dma_start(out=gt, in_=g2[:])
            pts = singles.tile([P, 1], F32)
            idt = singles.tile([P, E], F32)
            nc.vector.memset(pts, 0.0)
            nc.vector.memset(idt, 0.0)
            for q in range(N_Q):
                nc.sync.dma_start(out=pts[32 * q:32 * q + E, :], in_=pt[:])
                nc.sync.dma_start(out=idt[32 * q:32 * q + E, :], in_=idn[:])

            def half_body(half):
                psA = psl.tile([P, 512], F32, tag="psA")
                psX = psx.tile([P, 512], F32, tag="psX")
                # phase 1: hh loads; A (hh@gh) and X1 (hh@gl*256)
                for ld in range(NCH // CQH):
                    ht = hp.tile([P, CQH, T_HALF], BF16, tag="ht")
                    nc.sync.dma_start(out=ht, in_=hh[half, :,
                                                     ld * CQH:(ld + 1) * CQH,
                                                     :])
                    for j in range(CQH):
                        c = ld * CQH + j
                        for q in range(N_Q):
                            sl = slice(32 * q, 32 * q + E)
                            rhs = ht[:, j, q * 512:(q + 1) * 512]
                            nc.tensor.matmul(
                                psA[sl, :], lhsT=gt[:, c, 0, :], rhs=rhs,
                                start=(c == 0), stop=(c == NCH - 1),
                                tile_position=(0, 32 * q),
                                skip_group_check=True)
                            nc.tensor.matmul(
                                psX[sl, :], lhsT=gt[:, c, 1, :], rhs=rhs,
                                start=(c == 0), stop=False,
                                tile_position=(0, 32 * q),
                                skip_group_check=True)
                # phase 2: fp8 loads; X2 (hl*256 @ gh) accumulates onto X1
                for ld in range(NCH // CQL):
                    lt = lp.tile([P, CQL, T_HALF], F8, tag="lt")
                    nc.sync.dma_start(out=lt, in_=hl[half, :,
                                                     ld * CQL:(ld + 1) * CQL,
                                                     :])
                    for j in range(CQL):
                        c = ld * CQL + j
                        for q in range(N_Q):
                            sl = slice(32 * q, 32 * q + E)
                            nc.tensor.matmul(
                                psX[sl, :], lhsT=gt[:, c, 0, :],
                                rhs=lt[:, j, q * 512:(q + 1) * 512],
                                start=False, stop=(c == NCH - 1),
                                tile_position=(0, 32 * q),
                                skip_group_check=True)

                # logits = A + X*2^-8 + bias'   (full-bank reads: RAW deps
                # cover every col-group's accumulation in both banks)
                xs = ep.tile([P, 512], F32, tag="xs")
                nc.vector.tensor_scalar(
                    out=xs, in0=psX, scalar1=INV_LO, scalar2=None,
                    op0=mybir.AluOpType.mult)
                af1 = ep.tile([P, 512], F32, tag="af1")
                nc.vector.tensor_tensor(
                    out=af1, in0=psA, in1=xs, op=mybir.AluOpType.add)
                aff = ep.tile([P, 512], F32, tag="aff")
                nc.vector.tensor_scalar(
                    out=aff, in0=af1, scalar1=pts[:, 0:1], scalar2=None,
                    op0=mybir.AluOpType.add)

                tp = pst.tile([P, 512], F32, tag="tp")
                for q in range(N_Q):
                    sl = slice(32 * q, 32 * q + E)
                    aff_r = aff[sl, :].rearrange("e (k bj) -> e bj k",
                                                 bj=N_BJ)
                    for bj in range(N_BJ):
                        b = q * N_BJ + bj
                        nc.tensor.transpose(
                            tp[:, b * E:(b + 1) * E], aff_r[:, bj, :],
                            idt[sl, :], tile_position=(32 * q, 0))
                sc = ep.tile([P, N_BLK, E], F32, tag="sc")
                nc.vector.tensor_copy(out=sc, in_=tp[:, 0:N_BLK * E])

                mx = ep.tile([P, N_BLK, E], F32, tag="mx")
                mi = ep.tile([P, N_BLK, E], U32, tag="mi")
                for b in range(N_BLK):
                    nc.vector.max(out=mx[:, b, :], in_=sc[:, b, :])
                for b in range(N_BLK):
                    nc.vector.max_index(out=mi[:, b, :],
                                        in_max=mx[:, b, :],
                                        in_values=sc[:, b, :])

                dt_ = ep.tile([P, N_BLK], F32, tag="dt")
                nc.vector.tensor_tensor(
                    out=dt_, in0=mx[:, :, 1], in1=mx[:, :, 0],
                    op=mybir.AluOpType.subtract)
                et = ep.tile([P, N_BLK], F32, tag="et")
                nc.scalar.activation(
                    out=et, in_=dt_, func=mybir.ActivationFunctionType.Exp)
                st = ep.tile([P, N_BLK], F32, tag="st")
                nc.vector.tensor_scalar_add(st, et, 1.0)
                rt = ep.tile([P, N_BLK], F32, tag="rt")
                nc.vector.reciprocal(out=rt, in_=st)

                owt = ep.tile([P, N_BLK, 2], F32, tag="owt")
                nc.vector.tensor_copy(out=owt[:, :, 0], in_=rt)
                nc.vector.tensor_tensor(
                    out=owt[:, :, 1], in0=et, in1=rt,
                    op=mybir.AluOpType.mult)

                nc.sync.dma_start(
                    out=ow[half], in_=owt.rearrange(
                        "k (q bj) u -> k q bj u", q=N_Q))
                nc.sync.dma_start(
                    out=oe[half], in_=mi[:, :, 0:2].rearrange(
                        "k (q bj) u -> k q bj u", q=N_Q))

            def body():
                for half in range(2):
                    half_body(half)

            if n_rep == 1:
                body()
            else:
                with tc.For_i(0, n_rep, 1):
                    body()

    nc.finalize()
    return nc


def _get_nc():
    if "nc" not in _NC_CACHE:
        _NC_CACHE["nc"] = build_router_nc()
    return _NC_CACHE["nc"]


def make_gate_inputs(pressure_bias, temperature_field, gate_w):
    gw = np.asarray(gate_w, dtype=np.float32)
    pb = np.asarray(pressure_bias, np.float32)
    temp = np.asarray(temperature_field, np.float32)
    it = 1.0 / np.clip(temp, np.float32(0.1), np.float32(10.0))
    gs = gw * it[:, None]                                   # [E, D]
    gT = np.ascontiguousarray(gs.T)                         # [D, E]
    gh = gT.astype(ml_dtypes.bfloat16)
    gl = ((gT - gh.astype(np.float32)) * LO_SCALE).astype(ml_dtypes.bfloat16)
    g2 = np.stack([gh, gl], axis=1)                         # [D, 2, E]
    g2 = np.ascontiguousarray(
        g2.reshape(NCH, P, 2, E).transpose(1, 0, 2, 3))     # [P, NCH, 2, E]
    pt = np.ascontiguousarray((pb * it)[:, None])           # [E, 1]
    idn = np.eye(E, dtype=np.float32)
    return g2, pt, idn


def make_h_inputs(hs_core):
    """[T_CORE, D] fp32 -> hh [2, P, NCH, T_HALF] bf16,
    hl [2, P, NCH, T_HALF] fp8e4m3 (x256)."""
    hT = np.ascontiguousarray(hs_core.T)                    # [D, T] fp32
    hh_f = hT.astype(ml_dtypes.bfloat16)
    r = (hT - hh_f.astype(np.float32)) * np.float32(LO_SCALE)
    hl_f = r.astype(ml_dtypes.float8_e4m3)
    # [feat, tok] -> [half, p, c, t]: feat = c*P + p, tok = half*T_HALF + t
    hh_dev = np.ascontiguousarray(
        hh_f.reshape(NCH, P, 2, T_HALF).transpose(2, 1, 0, 3))
    hl_dev = np.ascontiguousarray(
        hl_f.reshape(NCH, P, 2, T_HALF).transpose(2, 1, 0, 3))
    return hh_dev, hl_dev


def unshuffle_out(arr, t_core):
    """[2, P, n_q, n_bj, u] -> [t_core, u]; t = half*2048 + q*512 + k*4 + bj"""
    return np.ascontiguousarray(
        arr.transpose(0, 2, 1, 3, 4).reshape(t_core, arr.shape[-1]))


def kernel(hidden_states, pressure_bias, temperature_field, gate_w):
    hs = np.ascontiguousarray(np.asarray(hidden_states, dtype=np.float32))
    hs = hs.reshape(T_TOTAL, D)
    g2, pt, idn = make_gate_inputs(pressure_bias, temperature_field, gate_w)

    in_maps = []
    for i in range(N_CORES):
        hh_dev, hl_dev = make_h_inputs(hs[i * T_CORE:(i + 1) * T_CORE])
        in_maps.append({"hh": hh_dev, "hl": hl_dev, "g2": g2,
                        "pt": pt, "idn": idn})

    nc = _get_nc()
    global LAST_RESULT
    res = run_bass_kernel_spmd(nc, in_maps, core_ids=list(range(N_CORES)),
                               trace=TRACE)
    LAST_RESULT = res

    weights = np.empty((T_TOTAL, 2), np.float32)
    experts = np.empty((T_TOTAL, 2), np.int32)
    for i, r in enumerate(res.results):
        weights[i * T_CORE:(i + 1) * T_CORE] = unshuffle_out(r["ow"], T_CORE)
        experts[i * T_CORE:(i + 1) * T_CORE] = (
            unshuffle_out(r["oe"], T_CORE).astype(np.int32))

    return weights.reshape(B, S, 2), experts.reshape(B, S, 2)


# revision 9
# speedup vs baseline: 7.7045x; 1.2256x over previous
"""v5: v4 + two PE-stream cuts.

  * A and X1 fused: one matmul per (chunk, col-group) with the 16-wide
    stationary [gh | gl*256] -> out [16, 512] (A in rows 0:8, X1*256 in
    rows 8:16 of the same bank).  512 GEMM matmuls instead of 768.
  * The descale/combine moves into the transpose stage: plain matmuls
    with a [16,8] matrix M = [I; 2^-8*I] contract the 16 A|X1 rows into
    transposed-and-combined [token, expert] tiles, and a second
    accumulating matmul adds X2 via 2^-8*I.  Bias is added by the DVE
    copy out of the transpose bank (tensor_tensor with a replicated
    bias tile).
  * hh streamed in 8 MiB loads.
"""

import numpy as np
import ml_dtypes

import concourse.bass as bass
import concourse.tile as tile
from concourse import bacc, mybir
from concourse.bass_utils import run_bass_kernel_spmd

F32 = mybir.dt.float32
BF16 = mybir.dt.bfloat16
F8 = mybir.dt.float8e4
U32 = mybir.dt.uint32

N_CORES = 8
B, S, D, E = 4, 8192, 4096, 8
T_TOTAL = B * S
T_CORE = T_TOTAL // N_CORES
P = 128
NCH = D // P                       # 32
T_HALF = T_CORE // 2               # 2048
N_Q = T_HALF // 512                # 4
N_BJ = 4
N_BLK = N_Q * N_BJ                 # 16
CQH = 16                           # bf16 chunks per DMA (8 MiB)
CQL = 16                           # fp8 chunks per DMA (4 MiB)
LO_SCALE = 256.0
INV_LO = 1.0 / LO_SCALE

_NC_CACHE = {}

TRACE = False
LAST_RESULT = None


def build_router_nc(n_rep=1, hbufs=2, lbufs=2):
    nc = bacc.Bacc(None, target_bir_lowering=False)

    hh = nc.dram_tensor("hh", [2, P, NCH, T_HALF], BF16,
                        kind="ExternalInput")
    hl = nc.dram_tensor("hl", [2, P, NCH, T_HALF], F8, kind="ExternalInput")
    g2 = nc.dram_tensor("g2", [P, NCH, 2 * E], BF16, kind="ExternalInput")
    # combine matrices: rows 0:16 = [I; 2^-8 I] (A|X1), rows 16:24 = 2^-8 I
    mm = nc.dram_tensor("mm", [3 * E, E], F32, kind="ExternalInput")
    bt = nc.dram_tensor("bt", [P, N_BLK, E], F32, kind="ExternalInput")
    ow = nc.dram_tensor("ow", [2, P, N_Q, N_BJ, 2], F32,
                        kind="ExternalOutput")
    oe = nc.dram_tensor("oe", [2, P, N_Q, N_BJ, 2], U32,
                        kind="ExternalOutput")

    with tile.TileContext(nc) as tc:
        with (
            tc.tile_pool(name="singles", bufs=1) as singles,
            tc.tile_pool(name="hp", bufs=hbufs) as hp,
            tc.tile_pool(name="lp", bufs=lbufs) as lp,
            tc.tile_pool(name="big", bufs=1) as big,
            tc.tile_pool(name="ep", bufs=2) as ep,
            tc.tile_pool(name="psl", bufs=2, space="PSUM") as psl,
            tc.tile_pool(name="psx", bufs=2, space="PSUM") as psx,
            tc.tile_pool(name="pst", bufs=2, space="PSUM") as pst,
        ):
            gt = singles.tile([P, NCH, 2 * E], BF16)
            nc.sync.dma_start(out=gt, in_=g2[:])
            btile = singles.tile([P, N_BLK, E], F32)
            nc.sync.dma_start(out=btile, in_=bt[:])
            # mA: [I; 2^-8 I] at rows 32q..32q+16; mB: 2^-8 I at 32q..32q+8
            mA = singles.tile([P, E], F32)
            mB = singles.tile([P, E], F32)
            nc.vector.memset(mA, 0.0)
            nc.vector.memset(mB, 0.0)
            for q in range(N_Q):
                nc.sync.dma_start(out=mA[32 * q:32 * q + 2 * E, :],
                                  in_=mm[0:2 * E, :])
                nc.sync.dma_start(out=mB[32 * q:32 * q + E, :],
                                  in_=mm[2 * E:3 * E, :])

            def half_body(half):
                psA = psl.tile([P, 512], F32, tag="psA")   # A | X1*256
                psX = psx.tile([P, 512], F32, tag="psX")   # X2*256
                for ld in range(NCH // CQH):
                    ht = hp.tile([P, CQH, T_HALF], BF16, tag="ht")
                    nc.sync.dma_start(
                        out=ht, in_=hh[half, :, ld * CQH:(ld + 1) * CQH, :])
                    for j in range(CQH):
                        c = ld * CQH + j
                        for q in range(N_Q):
                            nc.tensor.matmul(
                                psA[32 * q:32 * q + 2 * E, :],
                                lhsT=gt[:, c, :],
                                rhs=ht[:, j, q * 512:(q + 1) * 512],
                                start=(c == 0), stop=(c == NCH - 1),
                                tile_position=(0, 32 * q),
                                skip_group_check=True)
                for ld in range(NCH // CQL):
                    lt = lp.tile([P, CQL, T_HALF], F8, tag="lt")
                    nc.sync.dma_start(
                        out=lt, in_=hl[half, :, ld * CQL:(ld + 1) * CQL, :])
                    for j in range(CQL):
                        c = ld * CQL + j
                        for q in range(N_Q):
                            nc.tensor.matmul(
                                psX[32 * q:32 * q + E, :],
                                lhsT=gt[:, c, 0:E],
                                rhs=lt[:, j, q * 512:(q + 1) * 512],
                                start=(c == 0), stop=(c == NCH - 1),
                                tile_position=(0, 32 * q),
                                skip_group_check=True)

                # PSUM -> SBUF moves (PE matmuls read SBUF only; full-bank
                # reads give RAW deps on every col-group's accumulation)
                a16 = big.tile([P, 512], F32, tag="a16")
                nc.vector.tensor_copy(out=a16, in_=psA)
                x2s = big.tile([P, 512], F32, tag="x2s")
                nc.vector.tensor_copy(out=x2s, in_=psX)

                # combine-transpose: per block b=(q,bj), tokens {4k+bj}:
                #   tp[:, b] = a16[32q:32q+16]^T @ [I; 2^-8 I]
                #            + x2s[32q:32q+8]^T @ (2^-8 I)
                tp = pst.tile([P, 512], F32, tag="tp")
                for q in range(N_Q):
                    slA = slice(32 * q, 32 * q + 2 * E)
                    slX = slice(32 * q, 32 * q + E)
                    aR = a16[slA, :].rearrange("e (k bj) -> e bj k", bj=N_BJ)
                    xR = x2s[slX, :].rearrange("e (k bj) -> e bj k", bj=N_BJ)
                    for bj in range(N_BJ):
                        b = q * N_BJ + bj
                        nc.tensor.matmul(
                            tp[:, b * E:(b + 1) * E], lhsT=aR[:, bj, :],
                            rhs=mA[slA, :], start=True, stop=False,
                            tile_position=(32 * q, 0),
                            skip_group_check=True)
                        nc.tensor.matmul(
                            tp[:, b * E:(b + 1) * E], lhsT=xR[:, bj, :],
                            rhs=mB[slX, :], start=False, stop=True,
                            tile_position=(32 * q, 0),
                            skip_group_check=True)

                # sc = tp + bias (token-major; bias varies along free dim)
                sc = ep.tile([P, N_BLK, E], F32, tag="sc")
                nc.vector.tensor_tensor(
                    out=sc, in0=tp[:, 0:N_BLK * E].rearrange(
                        "p (b e) -> p b e", e=E),
                    in1=btile, op=mybir.AluOpType.add)

                mx = ep.tile([P, N_BLK, E], F32, tag="mx")
                mi = ep.tile([P, N_BLK, E], U32, tag="mi")
                for b in range(N_BLK):
                    nc.vector.max(out=mx[:, b, :], in_=sc[:, b, :])
                for b in range(N_BLK):
                    nc.vector.max_index(out=mi[:, b, :],
                                        in_max=mx[:, b, :],
                                        in_values=sc[:, b, :])

                dt_ = ep.tile([P, N_BLK], F32, tag="dt")
                nc.vector.tensor_tensor(
                    out=dt_, in0=mx[:, :, 1], in1=mx[:, :, 0],
                    op=mybir.AluOpType.subtract)
                et = ep.tile([P, N_BLK], F32, tag="et")
                nc.scalar.activation(
                    out=et, in_=dt_, func=mybir.ActivationFunctionType.Exp)
                st = ep.tile([P, N_BLK], F32, tag="st")
                nc.vector.tensor_scalar_add(st, et, 1.0)
                rt = ep.tile([P, N_BLK], F32, tag="rt")
                nc.vector.reciprocal(out=rt, in_=st)

                owt = ep.tile([P, N_BLK, 2], F32, tag="owt")
                nc.vector.tensor_copy(out=owt[:, :, 0], in_=rt)
                nc.vector.tensor_tensor(
                    out=owt[:, :, 1], in0=et, in1=rt,
                    op=mybir.AluOpType.mult)

                nc.sync.dma_start(
                    out=ow[half], in_=owt.rearrange(
                        "k (q bj) u -> k q bj u", q=N_Q))
                nc.sync.dma_start(
                    out=oe[half], in_=mi[:, :, 0:2].rearrange(
                        "k (q bj) u -> k q bj u", q=N_Q))

            def body():
                for half in range(2):
                    half_body(half)

            if n_rep == 1:
                body()
            else:
                with tc.For_i(0, n_rep, 1):
                    body()

    nc.finalize()
    return nc


def _get_nc():
    if "nc" not in _NC_CACHE:
        _NC_CACHE["nc"] = build_router_nc()
    return _NC_CACHE["nc"]


def make_gate_inputs(pressure_bias, temperature_field, gate_w):
    gw = np.asarray(gate_w, dtype=np.float32)
    pb = np.asarray(pressure_bias, np.float32)
    temp = np.asarray(temperature_field, np.float32)
    it = 1.0 / np.clip(temp, np.float32(0.1), np.float32(10.0))
    gs = gw * it[:, None]
    gT = np.ascontiguousarray(gs.T)                         # [D, E]
    gh = gT.astype(ml_dtypes.bfloat16)
    gl = ((gT - gh.astype(np.float32)) * LO_SCALE).astype(ml_dtypes.bfloat16)
    gcomb = np.concatenate([gh, gl], axis=1)                # [D, 16]
    g2 = np.ascontiguousarray(
        gcomb.reshape(NCH, P, 2 * E).transpose(1, 0, 2))    # [P, NCH, 16]
    eye = np.eye(E, dtype=np.float32)
    mm = np.concatenate([eye, eye * INV_LO, eye * INV_LO], axis=0)  # [24, 8]
    bias = (pb * it).astype(np.float32)
    bt = np.ascontiguousarray(np.broadcast_to(bias, (P, N_BLK, E)))
    return g2, mm, bt


def make_h_inputs(hs_core):
    hT = np.ascontiguousarray(hs_core.T)
    hh_f = hT.astype(ml_dtypes.bfloat16)
    r = (hT - hh_f.astype(np.float32)) * np.float32(LO_SCALE)
    hl_f = r.astype(ml_dtypes.float8_e4m3)
    hh_dev = np.ascontiguousarray(
        hh_f.reshape(NCH, P, 2, T_HALF).transpose(2, 1, 0, 3))
    hl_dev = np.ascontiguousarray(
        hl_f.reshape(NCH, P, 2, T_HALF).transpose(2, 1, 0, 3))
    return hh_dev, hl_dev


def unshuffle_out(arr, t_core):
    return np.ascontiguousarray(
        arr.transpose(0, 2, 1, 3, 4).reshape(t_core, arr.shape[-1]))


def kernel(hidden_states, pressure_bias, temperature_field, gate_w):
    hs = np.ascontiguousarray(np.asarray(hidden_states, dtype=np.float32))
    hs = hs.reshape(T_TOTAL, D)
    g2, mm, bt = make_gate_inputs(pressure_bias, temperature_field, gate_w)

    in_maps = []
    for i in range(N_CORES):
        hh_dev, hl_dev = make_h_inputs(hs[i * T_CORE:(i + 1) * T_CORE])
        in_maps.append({"hh": hh_dev, "hl": hl_dev, "g2": g2,
                        "mm": mm, "bt": bt})

    nc = _get_nc()
    global LAST_RESULT
    res = run_bass_kernel_spmd(nc, in_maps, core_ids=list(range(N_CORES)),
                               trace=TRACE)
    LAST_RESULT = res

    weights = np.empty((T_TOTAL, 2), np.float32)
    experts = np.empty((T_TOTAL, 2), np.int32)
    for i, r in enumerate(res.results):
        weights[i * T_CORE:(i + 1) * T_CORE] = unshuffle_out(r["ow"], T_CORE)
        experts[i * T_CORE:(i + 1) * T_CORE] = (
            unshuffle_out(r["oe"], T_CORE).astype(np.int32))

    return weights.reshape(B, S, 2), experts.reshape(B, S, 2)
